# revision 13
# baseline (speedup 1.0000x reference)
"""Causal multi-head attention (B=2, S=2048, D=1024, H=16) on 8 trn2 cores.

Sharding: core = (batch b = core//4, head-group g = core%4 of 4 heads).
Per core: Q/K/V projections for its 4 heads (Wq/Wk/Wv column-sharded),
causal attention, and the output projection against the row-shard of Wo.
The 4 per-batch partials are summed on the host (the TP all-reduce).

Layout strategy (no on-chip transposes anywhere):
  - activations arrive host-pre-transposed: xT (D, S), so Q^T/K^T
    projections land directly as (features, tokens) tiles.
  - scores are computed transposed, S^T (tk partitions, tq free), by
    lhsT=K^T rhs=Q^T. Heads are processed in pairs: each pair occupies
    partitions 0-63 / 64-127 of the Q^T/K^T tiles, and the two QK^T
    matmuls run row-tiled (concurrently) in the PE array.
  - P^T = exp(S^T/8) via ACT (one op covers both heads of a pair via a
    2-bank PSUM tile); causal masking = block skip + 4 diagonal masks
    applied on GPSIMD (otherwise idle).
  - PV uses V in natural layout (tokens, dk) augmented with a ones
    column, so the softmax denominator accumulates for free in PSUM
    row 64. out^T = PV^T lands as (features, tokens) == exactly the
    lhsT the output projection needs.
  - normalization: both heads' rowsum rows go to partitions {0,32} of
    one tile (ACT copies), one reciprocal, then a pick-DMA to DRAM and
    one broadcast-DMA materializes the (128, tq) reciprocal tile.
All matmuls run in float32r (full PE rate, ~1.5e-4 matmul rel err).
Emission is interleaved per tq-chunk (V first, then per chunk: Q/K
projection -> attention -> output projection) so DMA, PE, ACT, DVE
and GPSIMD phases overlap instead of serializing.
"""

import numpy as np

B, S, D, H = 2, 2048, 1024, 16
DK = D // H               # 64
N_CORES = 8
G = 4                     # head-groups (cores per batch)
HPG = H // G              # 4 heads per core
NPAIR = HPG // 2          # 2 head-pairs per core
E = HPG * DK              # 256 per-core projection width
TQ = 512                  # tq chunk (PSUM bank width in f32)
NQ = S // TQ              # 4 tq chunks
TK = 128                  # tk tile
NK = S // TK              # 16 tk tiles
KD = 128                  # contraction tile over D
NKD = D // KD             # 8

_NC_CACHE = None


def _build():
    import concourse.bass as bass
    import concourse.tile as tile
    from concourse import bacc, mybir

    F32 = mybir.dt.float32
    F32R = mybir.dt.float32r
    EXP = mybir.ActivationFunctionType.Exp

    nc = bacc.Bacc("TRN2", debug=False, num_devices=N_CORES)

    xqT = nc.dram_tensor("xqT", (D, S), F32R, kind="ExternalInput").ap()
    xkT = nc.dram_tensor("xkT", (D, S), F32R, kind="ExternalInput").ap()
    xvT = nc.dram_tensor("xvT", (D, S), F32R, kind="ExternalInput").ap()
    wql = nc.dram_tensor("wql", (D, E), F32R, kind="ExternalInput").ap()
    wkl = nc.dram_tensor("wkl", (D, E), F32R, kind="ExternalInput").ap()
    wvr = nc.dram_tensor("wvr", (D, E), F32R, kind="ExternalInput").ap()
    wor = nc.dram_tensor("wor", (E, D), F32R, kind="ExternalInput").ap()
    mask4 = nc.dram_tensor("mask4", (4, TK, TQ), F32, kind="ExternalInput").ap()
    ones_in = nc.dram_tensor("ones_in", (128, NK, HPG, 1), F32R,
                             kind="ExternalInput").ap()
    out = nc.dram_tensor("out", (S, D), F32, kind="ExternalOutput").ap()

    with tile.TileContext(nc) as tc:
        with tc.tile_pool(name="consts", bufs=1) as consts, \
             tc.tile_pool(name="stage", bufs=3) as stage, \
             tc.tile_pool(name="ppool", bufs=4) as ppool, \
             tc.tile_pool(name="norm", bufs=2) as norm, \
             tc.tile_pool(name="osb", bufs=2) as osb_pool, \
             tc.tile_pool(name="dr", bufs=4, space="DRAM") as dr, \
             tc.tile_pool(name="psum", bufs=1, space="PSUM") as psum:

            # ---- persistent SBUF; wvr + first xv chunks first so PE can
            # start the V projection as early as possible ----
            wvr_sb = consts.tile([128, NKD, E], F32R)
            nc.sync.dma_start(wvr_sb[:], wvr.rearrange("(k p) e -> p k e", p=128))
            vaug = consts.tile([128, NK, HPG, DK + 1], F32R)
            nc.sync.dma_start(vaug[:, :, :, DK:DK + 1], ones_in[:])

            wql_sb = consts.tile([128, NKD, E], F32R)
            wkl_sb = consts.tile([128, NKD, E], F32R)
            wor_sb = consts.tile([128, 2, D], F32R)
            mask_sb = consts.tile([128, 4, TQ], F32)

            qT_sb = [consts.tile([128, S], F32R, name=f"qT{j}") for j in range(NPAIR)]
            kT_sb = [consts.tile([128, S], F32R, name=f"kT{j}") for j in range(NPAIR)]
            oT_sb = [consts.tile([128, S], F32R, name=f"oT{j}") for j in range(NPAIR)]

            xq_r = xqT.rearrange("(k p) t -> p k t", p=128)
            xk_r = xkT.rearrange("(k p) t -> p k t", p=128)
            xv_r = xvT.rearrange("(k p) t -> p k t", p=128)

            # ---- V projection: natural layout (tokens, dk) per head ----
            def emit_v(mm):
                vs = stage.tile([128, NKD, 2 * TK], F32R, name=f"xv_{mm}",
                                tag="xv", bufs=2)
                nc.sync.dma_start(vs[:], xv_r[:, :, mm * 2 * TK:(mm + 1) * 2 * TK])
                for dm in range(2):
                    m = 2 * mm + dm
                    vp = psum.tile([128, E], F32, name=f"vp_{m}", tag="s", bufs=2)
                    for k in range(NKD):
                        nc.tensor.matmul(
                            vp[:], vs[:, k, dm * TK:(dm + 1) * TK], wvr_sb[:, k, :],
                            start=(k == 0), stop=(k == NKD - 1),
                        )
                    for h in range(HPG):
                        nc.vector.tensor_copy(
                            vaug[:, m, h, 0:DK], vp[:, h * DK:(h + 1) * DK])

            # ---- Q^T / K^T projection for one tq chunk ----
            def emit_qk(name, x_r, w_sb, dst, n):
                xs = [stage.tile([128, NKD // 2, TQ], F32R,
                                 name=f"x_{name}_{n}_{h}", tag="xstage")
                      for h in range(2)]
                for h in range(2):
                    nc.sync.dma_start(
                        xs[h][:],
                        x_r[:, h * (NKD // 2):(h + 1) * (NKD // 2),
                            n * TQ:(n + 1) * TQ])
                for j in range(NPAIR):
                    pp = psum.tile([128, TQ], F32, name=f"pp_{name}_{n}_{j}",
                                   tag="s", bufs=2)
                    for k in range(NKD):
                        nc.tensor.matmul(
                            pp[:],
                            w_sb[:, k, j * 128:(j + 1) * 128],
                            xs[k // 4][:, k % 4, :],
                            start=(k == 0), stop=(k == NKD - 1),
                        )
                    nc.vector.tensor_copy(dst[j][:, n * TQ:(n + 1) * TQ], pp[:])

            # ---- attention for one (pair, tq chunk) ----
            def emit_attention(j, n):
                pv = [
                    psum.tile([DK + 1, TQ], F32, name=f"pv_{j}_{n}_{hh}",
                              tag="pv", bufs=2)
                    for hh in range(2)
                ]
                n_tiles = 4 * n + 4
                for i in range(n_tiles):
                    o = i - 4 * n  # >= 0 only on diagonal tiles
                    f0 = max(0, o * TK)   # first valid tq col in chunk
                    s2 = psum.tile([128, 2 * TQ], F32, name=f"s_{j}_{n}_{i}",
                                   tag="s2", bufs=2)
                    for hh in range(2):
                        nc.tensor.matmul(
                            s2[:, hh * TQ + f0: (hh + 1) * TQ],
                            kT_sb[j][hh * 64:(hh + 1) * 64, i * TK:(i + 1) * TK],
                            qT_sb[j][hh * 64:(hh + 1) * 64, n * TQ + f0:(n + 1) * TQ],
                            start=True, stop=True,
                        )
                    p2 = ppool.tile([128, 2 * TQ], F32R, name=f"p_{j}_{n}_{i}",
                                    tag="p")
                    if f0 == 0:
                        nc.scalar.activation(p2[:], s2[:], EXP, scale=0.125)
                    else:
                        w = TQ - f0
                        src = bass.AP(
                            tensor=s2.tensor, offset=s2[:, f0:].offset,
                            ap=[list(s2.ap[0]), [TQ, 2], [1, w]],
                        )
                        dst = bass.AP(
                            tensor=p2.tensor, offset=p2[:, f0:].offset,
                            ap=[list(p2.ap[0]), [TQ, 2], [1, w]],
                        )
                        nc.scalar.activation(dst, src, EXP, scale=0.125)
                    if o >= 0:
                        for hh in range(2):
                            nc.gpsimd.tensor_mul(
                                p2[:, hh * TQ + f0:(hh + 1) * TQ],
                                p2[:, hh * TQ + f0:(hh + 1) * TQ],
                                mask_sb[:, o, f0:TQ],
                            )
                    for hh in range(2):
                        nc.tensor.matmul(
                            pv[hh][:, f0:TQ],
                            vaug[:, i, 2 * j + hh, :],
                            p2[:, hh * TQ + f0:(hh + 1) * TQ],
                            start=(i == 0), stop=(i == n_tiles - 1),
                        )
                # normalize by the ones-column rowsum, write oT.
                # Copy PSUM->SBUF first so the pv banks free quickly; copy
                # both rowsum rows to partitions {0,32} of one tile, one
                # reciprocal, then pick-DMA to DRAM + one broadcast-DMA.
                pvsb = [norm.tile([DK + 1, TQ], F32,
                                  name=f"pvsb_{j}_{n}_{hh}", tag=f"pvsb{hh}")
                        for hh in range(2)]
                rs2 = norm.tile([33, TQ], F32, name=f"rs2_{j}_{n}", tag="rs2")
                for hh in range(2):
                    nc.vector.tensor_copy(pvsb[hh][:], pv[hh][:])
                    nc.scalar.copy(rs2[32 * hh:32 * hh + 1, :],
                                   pv[hh][DK:DK + 1, :])
                rc2 = norm.tile([33, TQ], F32, name=f"rc2_{j}_{n}", tag="rc2")
                nc.vector.reciprocal_approx_fast(rc2[:], rs2[:])
                rcd = dr.tile([2, TQ], F32, name=f"rcd_{j}_{n}", tag="rcd")
                nc.sync.dma_start(
                    rcd[:],
                    bass.AP(tensor=rc2.tensor, offset=rc2.offset,
                            ap=[[rc2.ap[0][0] * 32, 2], [1, TQ]]),
                )
                bc2 = norm.tile([128, TQ], F32, name=f"bc2_{j}_{n}", tag="bc2")
                nc.sync.dma_start(
                    bc2[:],
                    bass.AP(tensor=rcd.tensor, offset=rcd.offset,
                            ap=[[TQ, 2], [0, 64], [1, TQ]]),
                )
                for hh in range(2):
                    nc.vector.tensor_mul(
                        oT_sb[j][hh * 64:(hh + 1) * 64, n * TQ:(n + 1) * TQ],
                        pvsb[hh][0:DK, :],
                        bc2[hh * 64:(hh + 1) * 64, :],
                    )

            # ---- output projection for one token tile ----
            def emit_outproj(m):
                o_sb = osb_pool.tile([128, D], F32, name=f"osb_{m}", tag="osb")
                for c in range(2):
                    op = psum.tile([128, TQ], F32, name=f"op_{m}_{c}", tag="s",
                                   bufs=2)
                    for j in range(NPAIR):
                        nc.tensor.matmul(
                            op[:],
                            oT_sb[j][:, m * TK:(m + 1) * TK],
                            wor_sb[:, j, c * TQ:(c + 1) * TQ],
                            start=(j == 0), stop=(j == NPAIR - 1),
                        )
                    nc.vector.tensor_copy(o_sb[:, c * TQ:(c + 1) * TQ], op[:])
                nc.sync.dma_start(out[m * TK:(m + 1) * TK, :], o_sb[:])

            # ---- emission order: V first (it gates all attention),
            # remaining weights behind it, then all Q/K projection chunks
            # (DMA streams continuously), then attention in chunk-major
            # order (earliest-ready first), then the output projection.
            emit_v(0)
            emit_v(1)
            nc.sync.dma_start(wql_sb[:], wql.rearrange("(k p) e -> p k e", p=128))
            nc.sync.dma_start(wkl_sb[:], wkl.rearrange("(k p) e -> p k e", p=128))
            nc.sync.dma_start(mask_sb[:], mask4.rearrange("o p f -> p o f"))
            emit_qk("q", xq_r, wql_sb, qT_sb, 0)
            emit_qk("k", xk_r, wkl_sb, kT_sb, 0)
            for n in range(1, NQ):
                emit_v(2 * n)
                emit_v(2 * n + 1)
                emit_qk("q", xq_r, wql_sb, qT_sb, n)
                emit_qk("k", xk_r, wkl_sb, kT_sb, n)
                for j in range(NPAIR):
                    emit_attention(j, n - 1)
            nc.sync.dma_start(wor_sb[:], wor.rearrange("(j p) f -> p j f", p=128))
            for j in range(NPAIR):
                emit_attention(j, NQ - 1)
            for m in range(NK):
                emit_outproj(m)

    nc.compile()
    return nc


def _get_nc():
    global _NC_CACHE
    if _NC_CACHE is None:
        _NC_CACHE = _build()
    return _NC_CACHE


def kernel(query, key, value, mask, Wq, Wk, Wv, Wo):
    from concourse.bass_utils import run_bass_kernel_spmd

    query = np.asarray(query, dtype=np.float32)
    key = np.asarray(key, dtype=np.float32)
    value = np.asarray(value, dtype=np.float32)
    mask = np.asarray(mask)
    Wq = np.asarray(Wq, dtype=np.float32)
    Wk = np.asarray(Wk, dtype=np.float32)
    Wv = np.asarray(Wv, dtype=np.float32)
    Wo = np.asarray(Wo, dtype=np.float32)

    # 4 diagonal-offset masks (tk-local partition p, tq-chunk col f):
    # keep iff tk_global <= tq_global  <=>  f >= o*128 + p.
    m4 = np.empty((4, TK, TQ), dtype=np.float32)
    msub = np.asarray(mask[0, :TQ, :TQ] != 0, dtype=np.float32)  # (tq, tk)
    for o in range(4):
        m4[o] = msub[:, o * TK:(o + 1) * TK].T

    xT = {}
    for b in range(B):
        xT[("q", b)] = np.ascontiguousarray(query[b].T)
        xT[("k", b)] = np.ascontiguousarray(key[b].T)
        xT[("v", b)] = np.ascontiguousarray(value[b].T)

    in_maps = []
    for core in range(N_CORES):
        b, g = divmod(core, G)
        sl = slice(g * E, (g + 1) * E)
        in_maps.append({
            "xqT": xT[("q", b)],
            "xkT": xT[("k", b)],
            "xvT": xT[("v", b)],
            "wql": np.ascontiguousarray(Wq[sl, :].T),
            "wkl": np.ascontiguousarray(Wk[sl, :].T),
            "wvr": np.ascontiguousarray(Wv[sl, :].T),
            "wor": np.ascontiguousarray(Wo[:, sl].T),
            "mask4": m4,
            "ones_in": np.ones((128, NK, HPG, 1), dtype=np.float32),
        })

    nc = _get_nc()
    res = run_bass_kernel_spmd(nc, in_maps, core_ids=list(range(N_CORES)))

    out = np.zeros((B, S, D), dtype=np.float32)
    for core in range(N_CORES):
        out[core // G] += res.results[core]["out"]
    return out


# revision 14
# speedup vs baseline: 1.0016x; 1.0016x over previous
"""Causal multi-head attention (B=2, S=2048, D=1024, H=16) on 8 trn2 cores.

Sharding: core = (batch b = core//4, head-group g = core%4 of 4 heads).
Per core: Q/K/V projections for its 4 heads (Wq/Wk/Wv column-sharded),
causal attention, and the output projection against the row-shard of Wo.
The 4 per-batch partials are summed on the host (the TP all-reduce).

Layout strategy (no on-chip transposes anywhere):
  - activations arrive host-pre-transposed: xT (D, S), so Q^T/K^T
    projections land directly as (features, tokens) tiles.
  - scores are computed transposed, S^T (tk partitions, tq free), by
    lhsT=K^T rhs=Q^T. Heads are processed in pairs: each pair occupies
    partitions 0-63 / 64-127 of the Q^T/K^T tiles, and the two QK^T
    matmuls run row-tiled (concurrently) in the PE array.
  - P^T = exp(S^T/8) via ACT (one op covers both heads of a pair via a
    2-bank PSUM tile); causal masking = block skip + 4 diagonal masks
    applied on GPSIMD (otherwise idle).
  - PV uses V in natural layout (tokens, dk) augmented with a ones
    column, so the softmax denominator accumulates for free in PSUM
    row 64. out^T = PV^T lands as (features, tokens) == exactly the
    lhsT the output projection needs.
  - normalization: both heads' rowsum rows go to partitions {0,32} of
    one tile (ACT copies), one reciprocal, then a pick-DMA to DRAM and
    one broadcast-DMA materializes the (128, tq) reciprocal tile.
All matmuls run in float32r (full PE rate, ~1.5e-4 matmul rel err).
Emission is interleaved per tq-chunk (V first, then per chunk: Q/K
projection -> attention -> output projection) so DMA, PE, ACT, DVE
and GPSIMD phases overlap instead of serializing.
"""

import numpy as np

B, S, D, H = 2, 2048, 1024, 16
DK = D // H               # 64
N_CORES = 8
G = 4                     # head-groups (cores per batch)
HPG = H // G              # 4 heads per core
NPAIR = HPG // 2          # 2 head-pairs per core
E = HPG * DK              # 256 per-core projection width
TQ = 512                  # tq chunk (PSUM bank width in f32)
NQ = S // TQ              # 4 tq chunks
TK = 128                  # tk tile
NK = S // TK              # 16 tk tiles
KD = 128                  # contraction tile over D
NKD = D // KD             # 8

_NC_CACHE = None


def _build():
    import concourse.bass as bass
    import concourse.tile as tile
    from concourse import bacc, mybir

    F32 = mybir.dt.float32
    F32R = mybir.dt.float32r
    EXP = mybir.ActivationFunctionType.Exp

    nc = bacc.Bacc("TRN2", debug=False, num_devices=N_CORES)

    xqT = nc.dram_tensor("xqT", (D, S), F32R, kind="ExternalInput").ap()
    xkT = nc.dram_tensor("xkT", (D, S), F32R, kind="ExternalInput").ap()
    xvT = nc.dram_tensor("xvT", (D, S), F32R, kind="ExternalInput").ap()
    wql = nc.dram_tensor("wql", (D, E), F32R, kind="ExternalInput").ap()
    wkl = nc.dram_tensor("wkl", (D, E), F32R, kind="ExternalInput").ap()
    wvr = nc.dram_tensor("wvr", (D, E), F32R, kind="ExternalInput").ap()
    wor = nc.dram_tensor("wor", (E, D), F32R, kind="ExternalInput").ap()
    mask4 = nc.dram_tensor("mask4", (4, TK, TQ), F32, kind="ExternalInput").ap()
    ones_in = nc.dram_tensor("ones_in", (128, NK, HPG, 1), F32R,
                             kind="ExternalInput").ap()
    out = nc.dram_tensor("out", (S, D), F32, kind="ExternalOutput").ap()

    with tile.TileContext(nc) as tc:
        with tc.tile_pool(name="consts", bufs=1) as consts, \
             tc.tile_pool(name="stage", bufs=3) as stage, \
             tc.tile_pool(name="ppool", bufs=4) as ppool, \
             tc.tile_pool(name="norm", bufs=2) as norm, \
             tc.tile_pool(name="osb", bufs=2) as osb_pool, \
             tc.tile_pool(name="dr", bufs=4, space="DRAM") as dr, \
             tc.tile_pool(name="psum", bufs=1, space="PSUM") as psum:

            # ---- persistent SBUF; wvr + first xv chunks first so PE can
            # start the V projection as early as possible ----
            wvr_sb = consts.tile([128, NKD, E], F32R)
            nc.sync.dma_start(wvr_sb[:], wvr.rearrange("(k p) e -> p k e", p=128))
            vaug = consts.tile([128, NK, HPG, DK + 1], F32R)
            nc.sync.dma_start(vaug[:, :, :, DK:DK + 1], ones_in[:])

            wql_sb = consts.tile([128, NKD, E], F32R)
            wkl_sb = consts.tile([128, NKD, E], F32R)
            wor_sb = consts.tile([128, 2, D], F32R)
            mask_sb = consts.tile([128, 4, TQ], F32)

            qT_sb = [consts.tile([128, S], F32R, name=f"qT{j}") for j in range(NPAIR)]
            kT_sb = [consts.tile([128, S], F32R, name=f"kT{j}") for j in range(NPAIR)]
            oT_sb = [consts.tile([128, S], F32R, name=f"oT{j}") for j in range(NPAIR)]

            xq_r = xqT.rearrange("(k p) t -> p k t", p=128)
            xk_r = xkT.rearrange("(k p) t -> p k t", p=128)
            xv_r = xvT.rearrange("(k p) t -> p k t", p=128)

            # ---- V projection: natural layout (tokens, dk) per head ----
            def emit_v(mm):
                vs = stage.tile([128, NKD, 2 * TK], F32R, name=f"xv_{mm}",
                                tag="xv", bufs=2)
                nc.sync.dma_start(vs[:], xv_r[:, :, mm * 2 * TK:(mm + 1) * 2 * TK])
                for dm in range(2):
                    m = 2 * mm + dm
                    vp = psum.tile([128, E], F32, name=f"vp_{m}", tag="s", bufs=2)
                    for k in range(NKD):
                        nc.tensor.matmul(
                            vp[:], vs[:, k, dm * TK:(dm + 1) * TK], wvr_sb[:, k, :],
                            start=(k == 0), stop=(k == NKD - 1),
                        )
                    for h in range(HPG):
                        nc.vector.tensor_copy(
                            vaug[:, m, h, 0:DK], vp[:, h * DK:(h + 1) * DK])

            # ---- Q^T / K^T projection for one tq chunk ----
            def emit_qk(name, x_r, w_sb, dst, n):
                xs = [stage.tile([128, NKD // 2, TQ], F32R,
                                 name=f"x_{name}_{n}_{h}", tag="xstage")
                      for h in range(2)]
                for h in range(2):
                    nc.sync.dma_start(
                        xs[h][:],
                        x_r[:, h * (NKD // 2):(h + 1) * (NKD // 2),
                            n * TQ:(n + 1) * TQ])
                for j in range(NPAIR):
                    pp = psum.tile([128, TQ], F32, name=f"pp_{name}_{n}_{j}",
                                   tag="s", bufs=2)
                    for k in range(NKD):
                        nc.tensor.matmul(
                            pp[:],
                            w_sb[:, k, j * 128:(j + 1) * 128],
                            xs[k // 4][:, k % 4, :],
                            start=(k == 0), stop=(k == NKD - 1),
                        )
                    nc.vector.tensor_copy(dst[j][:, n * TQ:(n + 1) * TQ], pp[:])

            # ---- attention for one (pair, tq chunk) ----
            def emit_attention(j, n):
                pv = [
                    psum.tile([DK + 1, TQ], F32, name=f"pv_{j}_{n}_{hh}",
                              tag="pv", bufs=2)
                    for hh in range(2)
                ]
                n_tiles = 4 * n + 4
                for i in range(n_tiles):
                    o = i - 4 * n  # >= 0 only on diagonal tiles
                    f0 = max(0, o * TK)   # first valid tq col in chunk
                    s2 = psum.tile([128, 2 * TQ], F32, name=f"s_{j}_{n}_{i}",
                                   tag="s2", bufs=2)
                    for hh in range(2):
                        nc.tensor.matmul(
                            s2[:, hh * TQ + f0: (hh + 1) * TQ],
                            kT_sb[j][hh * 64:(hh + 1) * 64, i * TK:(i + 1) * TK],
                            qT_sb[j][hh * 64:(hh + 1) * 64, n * TQ + f0:(n + 1) * TQ],
                            start=True, stop=True,
                        )
                    p2 = ppool.tile([128, 2 * TQ], F32R, name=f"p_{j}_{n}_{i}",
                                    tag="p")
                    if f0 == 0:
                        nc.scalar.activation(p2[:], s2[:], EXP, scale=0.125)
                    else:
                        w = TQ - f0
                        src = bass.AP(
                            tensor=s2.tensor, offset=s2[:, f0:].offset,
                            ap=[list(s2.ap[0]), [TQ, 2], [1, w]],
                        )
                        dst = bass.AP(
                            tensor=p2.tensor, offset=p2[:, f0:].offset,
                            ap=[list(p2.ap[0]), [TQ, 2], [1, w]],
                        )
                        nc.scalar.activation(dst, src, EXP, scale=0.125)
                    if o >= 0:
                        for hh in range(2):
                            nc.gpsimd.tensor_mul(
                                p2[:, hh * TQ + f0:(hh + 1) * TQ],
                                p2[:, hh * TQ + f0:(hh + 1) * TQ],
                                mask_sb[:, o, f0:TQ],
                            )
                    for hh in range(2):
                        nc.tensor.matmul(
                            pv[hh][:, f0:TQ],
                            vaug[:, i, 2 * j + hh, :],
                            p2[:, hh * TQ + f0:(hh + 1) * TQ],
                            start=(i == 0), stop=(i == n_tiles - 1),
                        )
                # normalize by the ones-column rowsum, write oT.
                # Copy PSUM->SBUF first so the pv banks free quickly; copy
                # both rowsum rows to partitions {0,32} of one tile, one
                # reciprocal, then pick-DMA to DRAM + one broadcast-DMA.
                pvsb = norm.tile([128, TQ], F32, name=f"pvsb_{j}_{n}", tag="pvsb")
                rs2 = norm.tile([33, TQ], F32, name=f"rs2_{j}_{n}", tag="rs2")
                for hh in range(2):
                    nc.vector.tensor_copy(
                        pvsb[hh * 64:(hh + 1) * 64, :], pv[hh][0:DK, :])
                    nc.scalar.copy(rs2[32 * hh:32 * hh + 1, :],
                                   pv[hh][DK:DK + 1, :])
                rc2 = norm.tile([33, TQ], F32, name=f"rc2_{j}_{n}", tag="rc2")
                nc.vector.reciprocal_approx_fast(rc2[:], rs2[:])
                rcd = dr.tile([2, TQ], F32, name=f"rcd_{j}_{n}", tag="rcd")
                nc.sync.dma_start(
                    rcd[:],
                    bass.AP(tensor=rc2.tensor, offset=rc2.offset,
                            ap=[[rc2.ap[0][0] * 32, 2], [1, TQ]]),
                )
                bc2 = norm.tile([128, TQ], F32, name=f"bc2_{j}_{n}", tag="bc2")
                nc.sync.dma_start(
                    bc2[:],
                    bass.AP(tensor=rcd.tensor, offset=rcd.offset,
                            ap=[[TQ, 2], [0, 64], [1, TQ]]),
                )
                nc.vector.tensor_mul(
                    oT_sb[j][:, n * TQ:(n + 1) * TQ], pvsb[:], bc2[:])

            # ---- output projection for one token tile ----
            def emit_outproj(m):
                o_sb = osb_pool.tile([128, D], F32, name=f"osb_{m}", tag="osb")
                for c in range(2):
                    op = psum.tile([128, TQ], F32, name=f"op_{m}_{c}", tag="s",
                                   bufs=2)
                    for j in range(NPAIR):
                        nc.tensor.matmul(
                            op[:],
                            oT_sb[j][:, m * TK:(m + 1) * TK],
                            wor_sb[:, j, c * TQ:(c + 1) * TQ],
                            start=(j == 0), stop=(j == NPAIR - 1),
                        )
                    nc.vector.tensor_copy(o_sb[:, c * TQ:(c + 1) * TQ], op[:])
                nc.sync.dma_start(out[m * TK:(m + 1) * TK, :], o_sb[:])

            # ---- emission order: V first (it gates all attention),
            # remaining weights behind it, then all Q/K projection chunks
            # (DMA streams continuously), then attention in chunk-major
            # order (earliest-ready first), then the output projection.
            emit_v(0)
            emit_v(1)
            nc.sync.dma_start(wql_sb[:], wql.rearrange("(k p) e -> p k e", p=128))
            nc.sync.dma_start(wkl_sb[:], wkl.rearrange("(k p) e -> p k e", p=128))
            nc.sync.dma_start(mask_sb[:], mask4.rearrange("o p f -> p o f"))
            emit_qk("q", xq_r, wql_sb, qT_sb, 0)
            emit_qk("k", xk_r, wkl_sb, kT_sb, 0)
            for n in range(1, NQ):
                emit_v(2 * n)
                emit_v(2 * n + 1)
                emit_qk("q", xq_r, wql_sb, qT_sb, n)
                emit_qk("k", xk_r, wkl_sb, kT_sb, n)
                for j in range(NPAIR):
                    emit_attention(j, n - 1)
            nc.sync.dma_start(wor_sb[:], wor.rearrange("(j p) f -> p j f", p=128))
            for j in range(NPAIR):
                emit_attention(j, NQ - 1)
            for m in range(NK):
                emit_outproj(m)

    nc.compile()
    return nc


def _get_nc():
    global _NC_CACHE
    if _NC_CACHE is None:
        _NC_CACHE = _build()
    return _NC_CACHE


def kernel(query, key, value, mask, Wq, Wk, Wv, Wo):
    from concourse.bass_utils import run_bass_kernel_spmd

    query = np.asarray(query, dtype=np.float32)
    key = np.asarray(key, dtype=np.float32)
    value = np.asarray(value, dtype=np.float32)
    mask = np.asarray(mask)
    Wq = np.asarray(Wq, dtype=np.float32)
    Wk = np.asarray(Wk, dtype=np.float32)
    Wv = np.asarray(Wv, dtype=np.float32)
    Wo = np.asarray(Wo, dtype=np.float32)

    # 4 diagonal-offset masks (tk-local partition p, tq-chunk col f):
    # keep iff tk_global <= tq_global  <=>  f >= o*128 + p.
    m4 = np.empty((4, TK, TQ), dtype=np.float32)
    msub = np.asarray(mask[0, :TQ, :TQ] != 0, dtype=np.float32)  # (tq, tk)
    for o in range(4):
        m4[o] = msub[:, o * TK:(o + 1) * TK].T

    xT = {}
    for b in range(B):
        xT[("q", b)] = np.ascontiguousarray(query[b].T)
        xT[("k", b)] = np.ascontiguousarray(key[b].T)
        xT[("v", b)] = np.ascontiguousarray(value[b].T)

    in_maps = []
    for core in range(N_CORES):
        b, g = divmod(core, G)
        sl = slice(g * E, (g + 1) * E)
        in_maps.append({
            "xqT": xT[("q", b)],
            "xkT": xT[("k", b)],
            "xvT": xT[("v", b)],
            "wql": np.ascontiguousarray(Wq[sl, :].T),
            "wkl": np.ascontiguousarray(Wk[sl, :].T),
            "wvr": np.ascontiguousarray(Wv[sl, :].T),
            "wor": np.ascontiguousarray(Wo[:, sl].T),
            "mask4": m4,
            "ones_in": np.ones((128, NK, HPG, 1), dtype=np.float32),
        })

    nc = _get_nc()
    res = run_bass_kernel_spmd(nc, in_maps, core_ids=list(range(N_CORES)))

    out = np.zeros((B, S, D), dtype=np.float32)
    for core in range(N_CORES):
        out[core // G] += res.results[core]["out"]
    return out
